# revision 1
# baseline (speedup 1.0000x reference)
"""ConvGRU Trainium2 kernel.

video [B=2, T=16, C=128, H=64, W=64] f32; 1x1-conv GRU over T.
Sharding: data-parallel over (B x H/16) -> 8 cores, each core owns
P = 16*64 = 1024 pixels for all T; weights replicated.

Per core, per timestep (pixels on the free dim, channels on partitions):
    zr_pre = [Wzx@x + Wzh@h | Wrx@x + Wrh@h]      (PE, fp16 in / fp32 psum)
    z = sigmoid(zr_pre[:P] + bz); r = sigmoid(zr_pre[P:] + br)   (ACT)
    rh = r * h                                     (DVE)
    c = tanh(Whx@x + Whh@rh + bh)                  (PE + ACT)
    h32 += z * (c - h16); h16 = cast(h32)          (DVE, fp32 state)

Numerics: fp16 matmul inputs/gates, fp32 PSUM accum + fp32 h state.
Measured against the f32 jax reference this lands ~7.5e-4 scale-relative
absmax (numpy simulation of the exact same rounding schedule).
"""

import os
import sys

import numpy as np

B, T, C, H, W = 2, 16, 128, 64, 64
NCORES = 8
HQ = H // 4          # 16 rows of H per core (4 H-slices x 2 batches = 8 cores)
P = HQ * W           # 1024 pixels per core
G = 2                # pixel groups per step (independent recurrence chains)
PG = P // G          # 512 pixels per group
NT = PG // 512       # 512-wide matmul tiles per group

_PROG = None


def _ensure_paths():
    for p in ("/opt/trn_rl_repo",):
        if p not in sys.path and os.path.isdir(p):
            sys.path.append(p)


def _build():
    _ensure_paths()
    import concourse.bacc as bacc
    import concourse.tile as tile
    from concourse import mybir

    f32 = mybir.dt.float32
    f16 = mybir.dt.float16
    AF = mybir.ActivationFunctionType

    nc = bacc.Bacc(
        "TRN2", target_bir_lowering=False, debug=False, num_devices=NCORES
    )
    x_dram = nc.dram_tensor("x_seq", [T, C, P], f16, kind="ExternalInput")
    w_dram = nc.dram_tensor("wmats", [6, C, C], f16, kind="ExternalInput")
    b_dram = nc.dram_tensor("biases", [C, 3], f32, kind="ExternalInput")
    o_dram = nc.dram_tensor("out_seq", [T, C, P], f32, kind="ExternalOutput")

    x_ap = x_dram.ap()
    w_ap = w_dram.ap()
    b_ap = b_dram.ap()
    o_ap = o_dram.ap()

    WZX, WZH, WRX, WRH, WHX, WHH = range(6)

    with tile.TileContext(nc) as tc:
        with (
            tc.tile_pool(name="consts", bufs=1) as consts,
            tc.tile_pool(name="xin", bufs=3) as xpool,
            tc.tile_pool(name="state", bufs=2) as spool,
            tc.tile_pool(name="work", bufs=2) as wk,
            tc.tile_pool(name="ps", bufs=1, space="PSUM") as ps,
        ):
            wt = consts.tile([C, 6 * C], f16)
            for i in range(6):
                nc.sync.dma_start(wt[:, i * C : (i + 1) * C], w_ap[i])
            bt = consts.tile([C, 3], f32)
            nc.sync.dma_start(bt[:], b_ap[:])

            def wslice(i):
                return wt[:, i * C : (i + 1) * C]

            # state per pixel group
            h32 = []
            h16 = []
            for g in range(G):
                t32 = spool.tile([C, PG], f32, tag=f"h32_{g}")
                nc.vector.memset(t32[:], 0.0)
                t16 = spool.tile([C, PG], f16, tag=f"h16_{g}")
                nc.vector.memset(t16[:], 0.0)
                h32.append(t32)
                h16.append(t16)

            for t in range(T):
                xg = []
                for g in range(G):
                    xt = xpool.tile([C, PG], f16, tag=f"x_{g}")
                    nc.sync.dma_start(xt[:], x_ap[t, :, g * PG : (g + 1) * PG])
                    xg.append(xt)

                for g in range(G):
                    x16 = xg[g]
                    hp32, hp16 = h32[g], h16[g]

                    # z|r pre-activations, one psum tile [C, 2*PG]
                    zr = ps.tile([C, 2 * PG], f32, tag=f"zr_{g}")
                    for n in range(NT):
                        s = slice(n * 512, (n + 1) * 512)
                        nc.tensor.matmul(
                            zr[:, s], wslice(WZX), x16[:, s],
                            start=True, stop=False,
                        )
                        nc.tensor.matmul(
                            zr[:, s], wslice(WZH), hp16[:, s],
                            start=False, stop=True,
                        )
                    for n in range(NT):
                        s = slice(n * 512, (n + 1) * 512)
                        so = slice(PG + n * 512, PG + (n + 1) * 512)
                        nc.tensor.matmul(
                            zr[:, so], wslice(WRX), x16[:, s],
                            start=True, stop=False,
                        )
                        nc.tensor.matmul(
                            zr[:, so], wslice(WRH), hp16[:, s],
                            start=False, stop=True,
                        )

                    r16 = wk.tile([C, PG], f16, tag=f"r_{g}")
                    nc.scalar.activation(
                        r16[:], zr[:, PG:], AF.Sigmoid, bias=bt[:, 1:2]
                    )
                    z16 = wk.tile([C, PG], f16, tag=f"z_{g}")
                    nc.scalar.activation(
                        z16[:], zr[:, :PG], AF.Sigmoid, bias=bt[:, 0:1]
                    )

                    rh16 = wk.tile([C, PG], f16, tag=f"rh_{g}")
                    nc.vector.tensor_mul(rh16[:], r16[:], hp16[:])

                    cp = ps.tile([C, PG], f32, tag=f"c_{g}")
                    for n in range(NT):
                        s = slice(n * 512, (n + 1) * 512)
                        nc.tensor.matmul(
                            cp[:, s], wslice(WHX), x16[:, s],
                            start=True, stop=False,
                        )
                        nc.tensor.matmul(
                            cp[:, s], wslice(WHH), rh16[:, s],
                            start=False, stop=True,
                        )
                    c16 = wk.tile([C, PG], f16, tag=f"c_{g}")
                    nc.scalar.activation(c16[:], cp[:], AF.Tanh, bias=bt[:, 2:3])

                    d16 = wk.tile([C, PG], f16, tag=f"d_{g}")
                    nc.vector.tensor_sub(d16[:], c16[:], hp16[:])
                    e32 = wk.tile([C, PG], f32, tag=f"e_{g}")
                    nc.vector.tensor_mul(e32[:], z16[:], d16[:])

                    n32 = spool.tile([C, PG], f32, tag=f"h32_{g}")
                    nc.vector.tensor_add(n32[:], hp32[:], e32[:])
                    n16 = spool.tile([C, PG], f16, tag=f"h16_{g}")
                    nc.vector.tensor_copy(n16[:], n32[:])
                    h32[g], h16[g] = n32, n16

                    nc.sync.dma_start(
                        o_ap[t, :, g * PG : (g + 1) * PG], n32[:]
                    )

    nc.compile()
    return nc


def _get_prog():
    global _PROG
    if _PROG is None:
        _PROG = _build()
    return _PROG


def kernel(video, Wz, bz, Wr, br, Wh, bh):
    _ensure_paths()
    from concourse.bass_utils import run_bass_kernel_spmd

    video = np.asarray(video, dtype=np.float32)
    nc = _get_prog()

    w6 = np.stack(
        [
            Wz[:, :C].T, Wz[:, C:].T,
            Wr[:, :C].T, Wr[:, C:].T,
            Wh[:, :C].T, Wh[:, C:].T,
        ]
    ).astype(np.float16)
    b3 = np.stack([bz, br, bh], axis=1).astype(np.float32)

    in_maps = []
    for core in range(NCORES):
        b_, q = divmod(core, 4)
        xs = np.ascontiguousarray(
            video[b_, :, :, q * HQ : (q + 1) * HQ, :]
        ).reshape(T, C, P).astype(np.float16)
        in_maps.append({"x_seq": xs, "wmats": w6, "biases": b3})

    res = run_bass_kernel_spmd(nc, in_maps, list(range(NCORES)))

    out = np.empty((B, T, C, H, W), np.float32)
    for core in range(NCORES):
        b_, q = divmod(core, 4)
        out[b_, :, :, q * HQ : (q + 1) * HQ, :] = np.asarray(
            res.results[core]["out_seq"]
        ).reshape(T, C, HQ, W)
    return out


# revision 2
# speedup vs baseline: 1.0998x; 1.0998x over previous
"""ConvGRU Trainium2 kernel.

video [B=2, T=16, C=128, H=64, W=64] f32; 1x1-conv GRU over T.
Sharding: data-parallel over (B x H/16) -> 8 cores, each core owns
P = 16*64 = 1024 pixels for all T; weights replicated.

Per core, per timestep (pixels on the free dim, channels on partitions):
    zr_pre = [Wzx@x + Wzh@h | Wrx@x + Wrh@h]      (PE, fp16 in / fp32 psum)
    z = sigmoid(zr_pre[:P] + bz); r = sigmoid(zr_pre[P:] + br)   (ACT)
    rh = r * h                                     (DVE)
    c = tanh(Whx@x + Whh@rh + bh)                  (PE + ACT)
    h32 += z * (c - h16); h16 = cast(h32)          (DVE, fp32 state)

The x-side matmul contributions for step t+1 are issued into step t's
serial tail (they only depend on the t+1 x DMA), keeping the PE dense so
the HAM clock gate stays at 8/8.

Numerics: fp16 matmul inputs/gates, fp32 PSUM accum + fp32 h state.
"""

import os
import sys

import numpy as np

B, T, C, H, W = 2, 16, 128, 64, 64
NCORES = 8
HQ = H // 4          # 16 rows of H per core (4 H-slices x 2 batches = 8 cores)
P = HQ * W           # 1024 pixels per core
G = 2                # pixel groups per step (independent recurrence chains)
PG = P // G          # 512 pixels per group

FP32_STATE = True    # keep h in fp32 (accuracy); False = pure fp16 state

_PROG = None


def _ensure_paths():
    for p in ("/opt/trn_rl_repo",):
        if p not in sys.path and os.path.isdir(p):
            sys.path.append(p)


def _build():
    _ensure_paths()
    import concourse.bacc as bacc
    import concourse.tile as tile
    from concourse import mybir

    f32 = mybir.dt.float32
    f16 = mybir.dt.float16
    AF = mybir.ActivationFunctionType

    nc = bacc.Bacc(
        "TRN2", target_bir_lowering=False, debug=False, num_devices=NCORES
    )
    x_dram = nc.dram_tensor("x_seq", [T, C, P], f16, kind="ExternalInput")
    w_dram = nc.dram_tensor("wmats", [6, C, C], f16, kind="ExternalInput")
    b_dram = nc.dram_tensor("biases", [C, 3], f32, kind="ExternalInput")
    o_dram = nc.dram_tensor("out_seq", [T, C, P], f32, kind="ExternalOutput")

    x_ap = x_dram.ap()
    w_ap = w_dram.ap()
    b_ap = b_dram.ap()
    o_ap = o_dram.ap()

    WZX, WZH, WRX, WRH, WHX, WHH = range(6)

    with tile.TileContext(nc) as tc:
        with (
            tc.tile_pool(name="consts", bufs=1) as consts,
            tc.tile_pool(name="xin", bufs=3) as xpool,
            tc.tile_pool(name="state", bufs=2) as spool,
            tc.tile_pool(name="work", bufs=2) as wk,
            tc.tile_pool(name="ps", bufs=1, space="PSUM") as ps,
        ):
            wt = consts.tile([C, 6 * C], f16)
            for i in range(6):
                nc.sync.dma_start(wt[:, i * C : (i + 1) * C], w_ap[i])
            bt = consts.tile([C, 3], f32)
            nc.sync.dma_start(bt[:], b_ap[:])

            def wslice(i):
                return wt[:, i * C : (i + 1) * C]

            # state per pixel group
            h32, h16 = [], []
            for g in range(G):
                if FP32_STATE:
                    t32 = spool.tile([C, PG], f32, tag=f"h32_{g}")
                    nc.vector.memset(t32[:], 0.0)
                    h32.append(t32)
                t16 = spool.tile([C, PG], f16, tag=f"h16_{g}")
                nc.vector.memset(t16[:], 0.0)
                h16.append(t16)

            def load_x(t):
                xt = xpool.tile([C, P], f16, tag="x")
                nc.sync.dma_start(xt[:], x_ap[t])
                return xt

            def x_parts(t, xt):
                """Open all accumulation groups for step t with the x-side
                contributions (independent of the recurrent state)."""
                zr_t, cp_t = [], []
                for g in range(G):
                    xs = xt[:, g * PG : (g + 1) * PG]
                    zr = ps.tile([C, 2 * PG], f32, tag=f"zr_{g}")
                    nc.tensor.matmul(
                        zr[:, :PG], wslice(WZX), xs, start=True, stop=False
                    )
                    nc.tensor.matmul(
                        zr[:, PG:], wslice(WRX), xs, start=True, stop=False
                    )
                    cp = ps.tile([C, PG], f32, tag=f"c_{g}")
                    nc.tensor.matmul(
                        cp[:], wslice(WHX), xs, start=True, stop=False
                    )
                    zr_t.append(zr)
                    cp_t.append(cp)
                return zr_t, cp_t

            x_t = load_x(0)
            zr_t, cp_t = x_parts(0, x_t)

            for t in range(T):
                x_next = load_x(t + 1) if t + 1 < T else None
                zr_next = cp_next = None

                for g in range(G):
                    hp16 = h16[g]
                    zr, cp = zr_t[g], cp_t[g]

                    # close the z|r accumulations with the recurrent parts
                    nc.tensor.matmul(
                        zr[:, :PG], wslice(WZH), hp16[:], start=False, stop=True
                    )
                    nc.tensor.matmul(
                        zr[:, PG:], wslice(WRH), hp16[:], start=False, stop=True
                    )

                    r16 = wk.tile([C, PG], f16, tag=f"r_{g}")
                    nc.scalar.activation(
                        r16[:], zr[:, PG:], AF.Sigmoid, bias=bt[:, 1:2]
                    )
                    z16 = wk.tile([C, PG], f16, tag=f"z_{g}")
                    nc.scalar.activation(
                        z16[:], zr[:, :PG], AF.Sigmoid, bias=bt[:, 0:1]
                    )

                    rh16 = wk.tile([C, PG], f16, tag=f"rh_{g}")
                    nc.vector.tensor_mul(rh16[:], r16[:], hp16[:])

                    nc.tensor.matmul(
                        cp[:], wslice(WHH), rh16[:], start=False, stop=True
                    )
                    c16 = wk.tile([C, PG], f16, tag=f"c_{g}")
                    nc.scalar.activation(c16[:], cp[:], AF.Tanh, bias=bt[:, 2:3])

                    # issue next step's x-side matmuls between this group's
                    # PE work and the elementwise tail (keeps PE dense)
                    if g == G - 1 and x_next is not None:
                        zr_next, cp_next = x_parts(t + 1, x_next)

                    d16 = wk.tile([C, PG], f16, tag=f"d_{g}")
                    nc.vector.tensor_sub(d16[:], c16[:], hp16[:])
                    e16 = wk.tile([C, PG], f16, tag=f"e_{g}")
                    nc.vector.tensor_mul(e16[:], z16[:], d16[:])

                    if FP32_STATE:
                        hp32 = h32[g]
                        n32 = spool.tile([C, PG], f32, tag=f"h32_{g}")
                        nc.vector.tensor_add(n32[:], hp32[:], e16[:])
                        n16 = spool.tile([C, PG], f16, tag=f"h16_{g}")
                        nc.vector.tensor_copy(n16[:], n32[:])
                        h32[g], h16[g] = n32, n16
                        nc.gpsimd.dma_start(
                            o_ap[t, :, g * PG : (g + 1) * PG], n32[:]
                        )
                    else:
                        n16 = spool.tile([C, PG], f16, tag=f"h16_{g}")
                        nc.vector.tensor_add(n16[:], hp16[:], e16[:])
                        h16[g] = n16
                        nc.gpsimd.dma_start(
                            o_ap[t, :, g * PG : (g + 1) * PG], n16[:]
                        )

                if x_next is not None:
                    x_t, zr_t, cp_t = x_next, zr_next, cp_next

    nc.compile()
    return nc


def _get_prog():
    global _PROG
    if _PROG is None:
        _PROG = _build()
    return _PROG


def _make_in_maps(video, Wz, bz, Wr, br, Wh, bh):
    w6 = np.stack(
        [
            Wz[:, :C].T, Wz[:, C:].T,
            Wr[:, :C].T, Wr[:, C:].T,
            Wh[:, :C].T, Wh[:, C:].T,
        ]
    ).astype(np.float16)
    b3 = np.stack([bz, br, bh], axis=1).astype(np.float32)
    in_maps = []
    for core in range(NCORES):
        b_, q = divmod(core, 4)
        xs = np.ascontiguousarray(
            video[b_, :, :, q * HQ : (q + 1) * HQ, :]
        ).reshape(T, C, P).astype(np.float16)
        in_maps.append({"x_seq": xs, "wmats": w6, "biases": b3})
    return in_maps


def kernel(video, Wz, bz, Wr, br, Wh, bh):
    _ensure_paths()
    from concourse.bass_utils import run_bass_kernel_spmd

    video = np.asarray(video, dtype=np.float32)
    nc = _get_prog()
    in_maps = _make_in_maps(video, Wz, bz, Wr, br, Wh, bh)
    res = run_bass_kernel_spmd(nc, in_maps, list(range(NCORES)))

    out = np.empty((B, T, C, H, W), np.float32)
    for core in range(NCORES):
        b_, q = divmod(core, 4)
        out[b_, :, :, q * HQ : (q + 1) * HQ, :] = np.asarray(
            res.results[core]["out_seq"]
        ).reshape(T, C, HQ, W)
    return out


# revision 3
# speedup vs baseline: 1.1055x; 1.0052x over previous
"""ConvGRU Trainium2 kernel.

video [B=2, T=16, C=128, H=64, W=64] f32; 1x1-conv GRU over T.
Sharding: data-parallel over (B x H/16) -> 8 cores, each core owns
P = 16*64 = 1024 pixels for all T; weights replicated.

Per core, per timestep (pixels on the free dim, channels on partitions):
    zr_pre = [Wzx@x + Wzh@h | Wrx@x + Wrh@h]      (PE, fp16 in / fp32 psum)
    z = sigmoid(zr_pre[:P] + bz); r = sigmoid(zr_pre[P:] + br)   (ACT)
    rh = r * h                                     (DVE)
    c = tanh(Whx@x + Whh@rh + bh)                  (PE + ACT)
    h32 += z * (c - h16); h16 = cast(h32)          (DVE, fp32 state)

The x-side matmul contributions for step t+1 are issued into step t's
serial tail (they only depend on the t+1 x DMA), keeping the PE dense so
the HAM clock gate stays at 8/8.

Numerics: fp16 matmul inputs/gates, fp32 PSUM accum + fp32 h state.
"""

import os
import sys

import numpy as np

B, T, C, H, W = 2, 16, 128, 64, 64
NCORES = 8
HQ = H // 4          # 16 rows of H per core (4 H-slices x 2 batches = 8 cores)
P = HQ * W           # 1024 pixels per core
G = 2                # pixel groups per step (independent recurrence chains)
PG = P // G          # 512 pixels per group

FP32_STATE = True    # keep h in fp32 (accuracy); False = pure fp16 state

_PROG = None


def _ensure_paths():
    for p in ("/opt/trn_rl_repo",):
        if p not in sys.path and os.path.isdir(p):
            sys.path.append(p)


def _build():
    _ensure_paths()
    import concourse.bacc as bacc
    import concourse.tile as tile
    from concourse import mybir

    f32 = mybir.dt.float32
    f16 = mybir.dt.float16
    AF = mybir.ActivationFunctionType

    nc = bacc.Bacc(
        "TRN2", target_bir_lowering=False, debug=False, num_devices=NCORES
    )
    x_dram = nc.dram_tensor("x_seq", [T, C, P], f16, kind="ExternalInput")
    w_dram = nc.dram_tensor("wmats", [6, C, C], f16, kind="ExternalInput")
    b_dram = nc.dram_tensor("biases", [C, 3], f32, kind="ExternalInput")
    o_dram = nc.dram_tensor("out_seq", [T, C, P], f32, kind="ExternalOutput")

    x_ap = x_dram.ap()
    w_ap = w_dram.ap()
    b_ap = b_dram.ap()
    o_ap = o_dram.ap()

    WZX, WZH, WRX, WRH, WHX, WHH = range(6)

    with tile.TileContext(nc) as tc:
        with (
            tc.tile_pool(name="consts", bufs=1) as consts,
            tc.tile_pool(name="xin", bufs=3) as xpool,
            tc.tile_pool(name="state", bufs=2) as spool,
            tc.tile_pool(name="work", bufs=2) as wk,
            tc.tile_pool(name="ps", bufs=1, space="PSUM") as ps,
        ):
            wt = consts.tile([C, 6 * C], f16)
            for i in range(6):
                nc.sync.dma_start(wt[:, i * C : (i + 1) * C], w_ap[i])
            bt = consts.tile([C, 3], f32)
            nc.sync.dma_start(bt[:], b_ap[:])

            def wslice(i):
                return wt[:, i * C : (i + 1) * C]

            # state per pixel group
            h32, h16 = [], []
            for g in range(G):
                if FP32_STATE:
                    t32 = spool.tile([C, PG], f32, tag=f"h32_{g}")
                    nc.vector.memset(t32[:], 0.0)
                    h32.append(t32)
                t16 = spool.tile([C, PG], f16, tag=f"h16_{g}")
                nc.vector.memset(t16[:], 0.0)
                h16.append(t16)

            def load_x(t):
                xt = xpool.tile([C, P], f16, tag="x")
                nc.sync.dma_start(xt[:], x_ap[t])
                return xt

            def open_zr(xt):
                """Open z|r accumulations with the x-side contributions."""
                zr_t = []
                for g in range(G):
                    xs = xt[:, g * PG : (g + 1) * PG]
                    zr = ps.tile([C, 2 * PG], f32, tag=f"zr_{g}", bufs=1)
                    nc.tensor.matmul(
                        zr[:, PG:], wslice(WRX), xs, start=True, stop=False
                    )
                    nc.tensor.matmul(
                        zr[:, :PG], wslice(WZX), xs, start=True, stop=False
                    )
                    zr_t.append(zr)
                return zr_t

            def open_c(xt):
                cp_t = []
                for g in range(G):
                    xs = xt[:, g * PG : (g + 1) * PG]
                    cp = ps.tile([C, PG], f32, tag=f"c_{g}", bufs=2)
                    nc.tensor.matmul(
                        cp[:], wslice(WHX), xs, start=True, stop=False
                    )
                    cp_t.append(cp)
                return cp_t

            x_t = load_x(0)
            zr_t = open_zr(x_t)
            cp_t = open_c(x_t)

            for t in range(T):
                x_next = load_x(t + 1) if t + 1 < T else None

                # -- PE: close the r then z accumulations (chain head) --
                for g in range(G):
                    nc.tensor.matmul(
                        zr_t[g][:, PG:], wslice(WRH), h16[g][:],
                        start=False, stop=True,
                    )
                for g in range(G):
                    nc.tensor.matmul(
                        zr_t[g][:, :PG], wslice(WZH), h16[g][:],
                        start=False, stop=True,
                    )

                # -- ACT: r sigmoids first (they gate rh -> c matmul) --
                r16 = []
                for g in range(G):
                    rt = wk.tile([C, PG], f16, tag=f"r_{g}")
                    nc.scalar.activation(
                        rt[:], zr_t[g][:, PG:], AF.Sigmoid, bias=bt[:, 1:2]
                    )
                    r16.append(rt)

                rh16 = []
                for g in range(G):
                    rh = wk.tile([C, PG], f16, tag=f"rh_{g}")
                    nc.vector.tensor_mul(rh[:], r16[g][:], h16[g][:])
                    rh16.append(rh)

                for g in range(G):
                    nc.tensor.matmul(
                        cp_t[g][:], wslice(WHH), rh16[g][:],
                        start=False, stop=True,
                    )

                # next step's c openers can run any time (double-buffered)
                cp_next = open_c(x_next) if x_next is not None else None

                # -- ACT: z0 t0 z1 t1 (z feeds only the late DVE blend) --
                z16, c16 = [None] * G, [None] * G
                for g in range(G):
                    zt = wk.tile([C, PG], f16, tag=f"z_{g}")
                    nc.scalar.activation(
                        zt[:], zr_t[g][:, :PG], AF.Sigmoid, bias=bt[:, 0:1]
                    )
                    z16[g] = zt
                    ct = wk.tile([C, PG], f16, tag=f"c16_{g}")
                    nc.scalar.activation(
                        ct[:], cp_t[g][:], AF.Tanh, bias=bt[:, 2:3]
                    )
                    c16[g] = ct

                # next step's z|r openers (wait on this step's sigmoids)
                zr_next = open_zr(x_next) if x_next is not None else None

                # -- DVE tail per group + output store --
                for g in range(G):
                    hp16 = h16[g]
                    d16 = wk.tile([C, PG], f16, tag=f"d_{g}")
                    nc.vector.tensor_sub(d16[:], c16[g][:], hp16[:])
                    e16 = wk.tile([C, PG], f16, tag=f"e_{g}")
                    nc.vector.tensor_mul(e16[:], z16[g][:], d16[:])

                    if FP32_STATE:
                        hp32 = h32[g]
                        n32 = spool.tile([C, PG], f32, tag=f"h32_{g}")
                        nc.vector.tensor_add(n32[:], hp32[:], e16[:])
                        n16 = spool.tile([C, PG], f16, tag=f"h16_{g}")
                        nc.vector.tensor_copy(n16[:], n32[:])
                        h32[g], h16[g] = n32, n16
                        nc.gpsimd.dma_start(
                            o_ap[t, :, g * PG : (g + 1) * PG], n32[:]
                        )
                    else:
                        n16 = spool.tile([C, PG], f16, tag=f"h16_{g}")
                        nc.vector.tensor_add(n16[:], hp16[:], e16[:])
                        h16[g] = n16
                        nc.gpsimd.dma_start(
                            o_ap[t, :, g * PG : (g + 1) * PG], n16[:]
                        )

                if x_next is not None:
                    x_t, zr_t, cp_t = x_next, zr_next, cp_next

    nc.compile()
    return nc


def _get_prog():
    global _PROG
    if _PROG is None:
        _PROG = _build()
    return _PROG


def _make_in_maps(video, Wz, bz, Wr, br, Wh, bh):
    w6 = np.stack(
        [
            Wz[:, :C].T, Wz[:, C:].T,
            Wr[:, :C].T, Wr[:, C:].T,
            Wh[:, :C].T, Wh[:, C:].T,
        ]
    ).astype(np.float16)
    b3 = np.stack([bz, br, bh], axis=1).astype(np.float32)
    in_maps = []
    for core in range(NCORES):
        b_, q = divmod(core, 4)
        xs = np.ascontiguousarray(
            video[b_, :, :, q * HQ : (q + 1) * HQ, :]
        ).reshape(T, C, P).astype(np.float16)
        in_maps.append({"x_seq": xs, "wmats": w6, "biases": b3})
    return in_maps


def kernel(video, Wz, bz, Wr, br, Wh, bh):
    _ensure_paths()
    from concourse.bass_utils import run_bass_kernel_spmd

    video = np.asarray(video, dtype=np.float32)
    nc = _get_prog()
    in_maps = _make_in_maps(video, Wz, bz, Wr, br, Wh, bh)
    res = run_bass_kernel_spmd(nc, in_maps, list(range(NCORES)))

    out = np.empty((B, T, C, H, W), np.float32)
    for core in range(NCORES):
        b_, q = divmod(core, 4)
        out[b_, :, :, q * HQ : (q + 1) * HQ, :] = np.asarray(
            res.results[core]["out_seq"]
        ).reshape(T, C, HQ, W)
    return out


# revision 4
# speedup vs baseline: 1.3411x; 1.2132x over previous
"""ConvGRU Trainium2 kernel.

video [B=2, T=16, C=128, H=64, W=64] f32; 1x1-conv GRU over T.
Sharding: data-parallel over (B x H/16) -> 8 cores, each core owns
P = 16*64 = 1024 pixels for all T; weights replicated.

Per core, per timestep (pixels on the free dim, channels on partitions):
    zr_pre = [Wzx@x + Wzh@h | Wrx@x + Wrh@h]      (PE, fp16 in / fp32 psum)
    z = sigmoid(zr_pre[:P] + bz); r = sigmoid(zr_pre[P:] + br)   (ACT)
    rh = r * h                                     (DVE)
    c = tanh(Whx@x + Whh@rh + bh)                  (PE + ACT)
    h = h + z * (c - h)                            (DVE, fp16 state)

The recurrence is latency-bound: each pixel group's step is a serial
cross-engine chain (h -> Wrh matmul -> sigmoid -> r*h -> Whh matmul ->
tanh -> blend -> h').  Structure choices below all serve that chain:
  - x-side matmul contributions for step t+1 are issued into step t's
    tail (PSUM c-tiles double buffered) to keep the PE dense/warm
  - r-gate work goes first (it gates the tanh matmul); z sigmoids are
    slotted late (only needed by the final blend)
  - group priority alternates per step so the second group's queueing
    penalty averages out instead of compounding on one chain
  - warmup matmuls + an early dummy activation hide the HAM clock-gate
    ramp and the ACT table load behind the initial x DMA

Numerics: fp16 matmul inputs/gates/state, fp32 PSUM accum + fp32 bias.
"""

import os
import sys

import numpy as np

B, T, C, H, W = 2, 16, 128, 64, 64
NCORES = 8
HQ = H // 4          # 16 rows of H per core (4 H-slices x 2 batches = 8 cores)
P = HQ * W           # 1024 pixels per core
G = 2                # pixel groups per step (independent recurrence chains)
PG = P // G          # 512 pixels per group

_PROG = None


def _ensure_paths():
    for p in ("/opt/trn_rl_repo",):
        if p not in sys.path and os.path.isdir(p):
            sys.path.append(p)


def _build():
    _ensure_paths()
    import concourse.bacc as bacc
    import concourse.tile as tile
    from concourse import mybir

    f32 = mybir.dt.float32
    f16 = mybir.dt.float16
    AF = mybir.ActivationFunctionType

    nc = bacc.Bacc(
        "TRN2", target_bir_lowering=False, debug=False, num_devices=NCORES
    )
    x_dram = nc.dram_tensor("x_seq", [T, C, P], f16, kind="ExternalInput")
    w_dram = nc.dram_tensor("wmats", [C, 6 * C], f16, kind="ExternalInput")
    b_dram = nc.dram_tensor("biases", [C, 3], f32, kind="ExternalInput")
    o_dram = nc.dram_tensor("out_seq", [T, C, P], f32, kind="ExternalOutput")

    x_ap = x_dram.ap()
    w_ap = w_dram.ap()
    b_ap = b_dram.ap()
    o_ap = o_dram.ap()

    WZX, WZH, WRX, WRH, WHX, WHH = range(6)

    with tile.TileContext(nc) as tc:
        with (
            tc.tile_pool(name="consts", bufs=1) as consts,
            tc.tile_pool(name="xin", bufs=4) as xpool,
            tc.tile_pool(name="state", bufs=2) as spool,
            tc.tile_pool(name="work", bufs=2) as wk,
            tc.tile_pool(name="ps", bufs=1, space="PSUM") as ps,
        ):
            wt = consts.tile([C, 6 * C], f16)
            nc.sync.dma_start(wt[:], w_ap[:])
            bt = consts.tile([C, 3], f32)
            nc.sync.dma_start(bt[:], b_ap[:])

            def wslice(i):
                return wt[:, i * C : (i + 1) * C]

            # fp16 state per pixel group
            h16 = []
            for g in range(G):
                t16 = spool.tile([C, PG], f16, tag=f"h16_{g}")
                nc.vector.memset(t16[:], 0.0)
                h16.append(t16)

            # -- warmup: ramp the PE clock gate + preload the ACT table
            #    while the first x DMA is in flight --
            warm = ps.tile([C, PG], f32, tag="zr_0")
            for i in range(8):
                nc.tensor.matmul(
                    warm[:], wslice(i % 6), wt[:, :PG],
                    start=True, stop=True,
                )
            wtmp = wk.tile([C, PG], f16, tag="r_0")
            nc.scalar.activation(
                wtmp[:], warm[:], AF.Sigmoid, bias=bt[:, 0:1]
            )

            def load_x(t):
                xt = xpool.tile([C, P], f16, tag="x")
                nc.sync.dma_start(xt[:], x_ap[t])
                return xt

            def open_zr(xt, gorder):
                """Open z|r accumulations with the x-side contributions."""
                zr_t = [None] * G
                for g in gorder:
                    xs = xt[:, g * PG : (g + 1) * PG]
                    zr = ps.tile([C, 2 * PG], f32, tag=f"zr_{g}", bufs=1)
                    nc.tensor.matmul(
                        zr[:, PG:], wslice(WRX), xs, start=True, stop=False
                    )
                    nc.tensor.matmul(
                        zr[:, :PG], wslice(WZX), xs, start=True, stop=False
                    )
                    zr_t[g] = zr
                return zr_t

            def open_c(xt, gorder):
                cp_t = [None] * G
                for g in gorder:
                    xs = xt[:, g * PG : (g + 1) * PG]
                    cp = ps.tile([C, PG], f32, tag=f"c_{g}", bufs=2)
                    nc.tensor.matmul(
                        cp[:], wslice(WHX), xs, start=True, stop=False
                    )
                    cp_t[g] = cp
                return cp_t

            first = list(range(G))
            x_t = load_x(0)
            zr_t = open_zr(x_t, first)
            cp_t = open_c(x_t, first)

            for t in range(T):
                go = first if t % 2 == 0 else first[::-1]
                x_next = load_x(t + 1) if t + 1 < T else None

                # -- PE: close the r then z accumulations (chain head) --
                for g in go:
                    nc.tensor.matmul(
                        zr_t[g][:, PG:], wslice(WRH), h16[g][:],
                        start=False, stop=True,
                    )
                for g in go:
                    nc.tensor.matmul(
                        zr_t[g][:, :PG], wslice(WZH), h16[g][:],
                        start=False, stop=True,
                    )

                # -- ACT: r sigmoids first (they gate rh -> c matmul) --
                r16 = [None] * G
                for g in go:
                    rt = wk.tile([C, PG], f16, tag=f"r_{g}")
                    nc.scalar.activation(
                        rt[:], zr_t[g][:, PG:], AF.Sigmoid, bias=bt[:, 1:2]
                    )
                    r16[g] = rt

                rh16 = [None] * G
                for g in go:
                    rh = wk.tile([C, PG], f16, tag=f"rh_{g}")
                    nc.vector.tensor_mul(rh[:], r16[g][:], h16[g][:])
                    rh16[g] = rh

                for g in go:
                    nc.tensor.matmul(
                        cp_t[g][:], wslice(WHH), rh16[g][:],
                        start=False, stop=True,
                    )

                # next step's c openers can run any time (double-buffered)
                cp_next = open_c(x_next, go) if x_next is not None else None

                # -- ACT: z/tanh interleaved (z feeds only the late blend) --
                z16, c16 = [None] * G, [None] * G
                for g in go:
                    zt = wk.tile([C, PG], f16, tag=f"z_{g}")
                    nc.scalar.activation(
                        zt[:], zr_t[g][:, :PG], AF.Sigmoid, bias=bt[:, 0:1]
                    )
                    z16[g] = zt
                    ct = wk.tile([C, PG], f16, tag=f"c16_{g}")
                    nc.scalar.activation(
                        ct[:], cp_t[g][:], AF.Tanh, bias=bt[:, 2:3]
                    )
                    c16[g] = ct

                # next step's z|r openers (wait on this step's sigmoids)
                zr_next = open_zr(x_next, go) if x_next is not None else None

                # -- DVE tail per group + output store --
                for g in go:
                    hp16 = h16[g]
                    d16 = wk.tile([C, PG], f16, tag=f"d_{g}")
                    nc.vector.tensor_sub(d16[:], c16[g][:], hp16[:])
                    e16 = wk.tile([C, PG], f16, tag=f"e_{g}")
                    nc.vector.tensor_mul(e16[:], z16[g][:], d16[:])
                    n16 = spool.tile([C, PG], f16, tag=f"h16_{g}")
                    nc.vector.tensor_add(n16[:], hp16[:], e16[:])
                    h16[g] = n16
                    nc.gpsimd.dma_start(
                        o_ap[t, :, g * PG : (g + 1) * PG], n16[:]
                    )

                if x_next is not None:
                    x_t, zr_t, cp_t = x_next, zr_next, cp_next

    nc.compile()
    return nc


def _get_prog():
    global _PROG
    if _PROG is None:
        _PROG = _build()
    return _PROG


def _make_in_maps(video, Wz, bz, Wr, br, Wh, bh):
    w6 = np.concatenate(
        [
            Wz[:, :C].T, Wz[:, C:].T,
            Wr[:, :C].T, Wr[:, C:].T,
            Wh[:, :C].T, Wh[:, C:].T,
        ],
        axis=1,
    ).astype(np.float16)
    b3 = np.stack([bz, br, bh], axis=1).astype(np.float32)
    in_maps = []
    for core in range(NCORES):
        b_, q = divmod(core, 4)
        xs = np.ascontiguousarray(
            video[b_, :, :, q * HQ : (q + 1) * HQ, :]
        ).reshape(T, C, P).astype(np.float16)
        in_maps.append({"x_seq": xs, "wmats": w6, "biases": b3})
    return in_maps


def kernel(video, Wz, bz, Wr, br, Wh, bh):
    _ensure_paths()
    from concourse.bass_utils import run_bass_kernel_spmd

    video = np.asarray(video, dtype=np.float32)
    nc = _get_prog()
    in_maps = _make_in_maps(video, Wz, bz, Wr, br, Wh, bh)
    res = run_bass_kernel_spmd(nc, in_maps, list(range(NCORES)))

    out = np.empty((B, T, C, H, W), np.float32)
    for core in range(NCORES):
        b_, q = divmod(core, 4)
        out[b_, :, :, q * HQ : (q + 1) * HQ, :] = np.asarray(
            res.results[core]["out_seq"]
        ).reshape(T, C, HQ, W)
    return out


# revision 10
# speedup vs baseline: 1.4238x; 1.0616x over previous
"""ConvGRU Trainium2 kernel.

video [B=2, T=16, C=128, H=64, W=64] f32; 1x1-conv GRU over T.
Sharding: data-parallel over (B x H/16) -> 8 cores, each core owns
P = 16*64 = 1024 pixels for all T; weights replicated.

Per core, per timestep (pixels on the free dim, channels on partitions):
    zr_pre = [Wzx@x + Wzh@h | Wrx@x + Wrh@h]      (PE, fp16 in / fp32 psum)
    z = sigmoid(zr_pre[:P] + bz); r = sigmoid(zr_pre[P:] + br)   (ACT)
    rh = r * h                                     (DVE)
    c = tanh(Whx@x + Whh@rh + bh)                  (PE + ACT)
    h = h + z * (c - h)                            (DVE, fp16 state)

The recurrence is latency-bound: each pixel group's step is a serial
cross-engine chain (h -> Wrh matmul -> sigmoid -> r*h -> Whh matmul ->
tanh -> blend -> h').  Structure choices below all serve that chain:
  - x-side matmul contributions for step t+1 are issued into step t's
    tail (PSUM c-tiles double buffered) to keep the PE dense/warm
  - r-gate work goes first (it gates the tanh matmul); z sigmoids are
    slotted late (only needed by the final blend)
  - group priority alternates per step so the second group's queueing
    penalty averages out instead of compounding on one chain
  - warmup matmuls + an early dummy activation hide the HAM clock-gate
    ramp and the ACT table load behind the initial x DMA

Numerics: fp16 matmul inputs/gates/state, fp32 PSUM accum + fp32 bias.
"""

import os
import sys

import numpy as np

B, T, C, H, W = 2, 16, 128, 64, 64
NCORES = 8
HQ = H // 4          # 16 rows of H per core (4 H-slices x 2 batches = 8 cores)
P = HQ * W           # 1024 pixels per core
G = 2                # pixel groups per step (independent recurrence chains)
PG = P // G          # 512 pixels per group

_PROG = None


def _ensure_paths():
    for p in ("/opt/trn_rl_repo",):
        if p not in sys.path and os.path.isdir(p):
            sys.path.append(p)


def _build():
    _ensure_paths()
    import concourse.bacc as bacc
    import concourse.tile as tile
    from concourse import mybir

    f32 = mybir.dt.float32
    f16 = mybir.dt.float16
    AF = mybir.ActivationFunctionType

    nc = bacc.Bacc(
        "TRN2", target_bir_lowering=False, debug=False, num_devices=NCORES
    )
    x_dram = nc.dram_tensor("x_seq", [T, C, P], f16, kind="ExternalInput")
    w_dram = nc.dram_tensor("wmats", [C, 6 * C], f16, kind="ExternalInput")
    b_dram = nc.dram_tensor("biases", [C, 4], f32, kind="ExternalInput")
    o_dram = nc.dram_tensor("out_seq", [T, C, P], f16, kind="ExternalOutput")

    x_ap = x_dram.ap()
    w_ap = w_dram.ap()
    b_ap = b_dram.ap()
    o_ap = o_dram.ap()

    WZX, WZH, WRX, WRH, WHX, WHH = range(6)

    with tile.TileContext(nc) as tc:
        with (
            tc.tile_pool(name="consts", bufs=1) as consts,
            tc.tile_pool(name="xin", bufs=4) as xpool,
            tc.tile_pool(name="state", bufs=2) as spool,
            tc.tile_pool(name="work", bufs=2) as wk,
            tc.tile_pool(name="ps", bufs=1, space="PSUM") as ps,
        ):
            wt = consts.tile([C, 6 * C], f16)
            nc.sync.dma_start(wt[:], w_ap[:])
            bt = consts.tile([C, 4], f32)
            nc.gpsimd.dma_start(bt[:], b_ap[:])

            def wslice(i):
                return wt[:, i * C : (i + 1) * C]

            # fp16 state per pixel group
            h16 = []
            for g in range(G):
                t16 = spool.tile([C, PG], f16, tag=f"h16_{g}")
                nc.vector.memset(t16[:], 0.0)
                h16.append(t16)

            # -- warmup: ramp the PE clock gate + preload the ACT table
            #    while the first x DMA is in flight --
            warm = ps.tile([C, PG], f32, tag="zr_0")
            for i in range(5):
                nc.tensor.matmul(
                    warm[:], wslice(i % 6), wt[:, :PG],
                    start=True, stop=True,
                )
            wtmp = wk.tile([C, PG], f16, tag="r_0")
            nc.scalar.activation(
                wtmp[:], warm[:], AF.Sigmoid, bias=bt[:, 0:1]
            )

            def load_x(t):
                xt = xpool.tile([C, P], f16, tag="x")
                nc.sync.dma_start(xt[:], x_ap[t])
                return xt

            def open_zr(xt, gorder):
                """Open z|r accumulations with the x-side contributions."""
                zr_t = [None] * G
                for g in gorder:
                    xs = xt[:, g * PG : (g + 1) * PG]
                    zr = ps.tile([C, 2 * PG], f32, tag=f"zr_{g}", bufs=1)
                    nc.tensor.matmul(
                        zr[:, PG:], wslice(WRX), xs, start=True, stop=False
                    )
                    nc.tensor.matmul(
                        zr[:, :PG], wslice(WZX), xs, start=True, stop=False
                    )
                    zr_t[g] = zr
                return zr_t

            def open_c(xt, gorder):
                cp_t = [None] * G
                for g in gorder:
                    xs = xt[:, g * PG : (g + 1) * PG]
                    cp = ps.tile([C, PG], f32, tag=f"c_{g}", bufs=2)
                    nc.tensor.matmul(
                        cp[:], wslice(WHX), xs, start=True, stop=False
                    )
                    cp_t[g] = cp
                return cp_t

            first = list(range(G))
            x_t = load_x(0)
            zr_t = open_zr(x_t, first)
            cp_t = open_c(x_t, first)

            for t in range(T):
                go = first if t % 2 == 0 else first[::-1]
                x_next = load_x(t + 1) if t + 1 < T else None

                # -- PE: close the r then z accumulations (chain head) --
                for g in go:
                    nc.tensor.matmul(
                        zr_t[g][:, PG:], wslice(WRH), h16[g][:],
                        start=False, stop=True,
                    )
                for g in go:
                    nc.tensor.matmul(
                        zr_t[g][:, :PG], wslice(WZH), h16[g][:],
                        start=False, stop=True,
                    )

                # -- ACT: r sigmoids first (they gate rh -> c matmul) --
                r16 = [None] * G
                for g in go:
                    rt = wk.tile([C, PG], f16, tag=f"r_{g}")
                    nc.scalar.activation(
                        rt[:], zr_t[g][:, PG:], AF.Sigmoid, bias=bt[:, 1:2]
                    )
                    r16[g] = rt

                rh16 = [None] * G
                for g in go:
                    rh = wk.tile([C, PG], f16, tag=f"rh_{g}")
                    nc.vector.tensor_mul(rh[:], r16[g][:], h16[g][:])
                    rh16[g] = rh

                for g in go:
                    nc.tensor.matmul(
                        cp_t[g][:], wslice(WHH), rh16[g][:],
                        start=False, stop=True,
                    )

                # next step's c openers can run any time (double-buffered)
                cp_next = open_c(x_next, go) if x_next is not None else None

                # -- ACT: zbar/tanh interleaved; zbar = 1-z = sigmoid(-pre)
                #    feeds the blend h' = zbar*h + (1-zbar)*c, whose only
                #    post-tanh serial ops are v = z*c and h' = u + v --
                zb16, c16 = [None] * G, [None] * G
                for g in go:
                    zbt = wk.tile([C, PG], f16, tag=f"zb_{g}")
                    nc.scalar.activation(
                        zbt[:], zr_t[g][:, :PG], AF.Sigmoid,
                        bias=bt[:, 3:4], scale=-1.0,
                    )
                    zb16[g] = zbt
                    ct = wk.tile([C, PG], f16, tag=f"c16_{g}")
                    nc.scalar.activation(
                        ct[:], cp_t[g][:], AF.Tanh, bias=bt[:, 2:3]
                    )
                    c16[g] = ct

                # next step's z|r openers (wait on this step's sigmoids)
                zr_next = open_zr(x_next, go) if x_next is not None else None

                # -- DVE mid-chain: u = zbar*h and z = 1-zbar overlap the
                #    tanh; only v and the final add trail it --
                u16, z16 = [None] * G, [None] * G
                for g in go:
                    ut = wk.tile([C, PG], f16, tag=f"u_{g}")
                    nc.vector.tensor_mul(ut[:], zb16[g][:], h16[g][:])
                    u16[g] = ut
                    zt = wk.tile([C, PG], f16, tag=f"z_{g}")
                    nc.vector.tensor_scalar(
                        zt[:], zb16[g][:], -1.0, 1.0,
                        mybir.AluOpType.mult, mybir.AluOpType.add,
                    )
                    z16[g] = zt

                for g in go:
                    v16 = wk.tile([C, PG], f16, tag=f"v_{g}")
                    nc.vector.tensor_mul(v16[:], z16[g][:], c16[g][:])
                    n16 = spool.tile([C, PG], f16, tag=f"h16_{g}")
                    nc.vector.tensor_add(n16[:], u16[g][:], v16[:])
                    h16[g] = n16
                    nc.sync.dma_start(
                        o_ap[t, :, g * PG : (g + 1) * PG], n16[:]
                    )

                if x_next is not None:
                    x_t, zr_t, cp_t = x_next, zr_next, cp_next

    nc.compile()
    return nc


def _get_prog():
    global _PROG
    if _PROG is None:
        _PROG = _build()
    return _PROG


def _make_in_maps(video, Wz, bz, Wr, br, Wh, bh):
    w6 = np.concatenate(
        [
            Wz[:, :C].T, Wz[:, C:].T,
            Wr[:, :C].T, Wr[:, C:].T,
            Wh[:, :C].T, Wh[:, C:].T,
        ],
        axis=1,
    ).astype(np.float16)
    b3 = np.stack([bz, br, bh, -bz], axis=1).astype(np.float32)
    in_maps = []
    for core in range(NCORES):
        b_, q = divmod(core, 4)
        xs = np.ascontiguousarray(
            video[b_, :, :, q * HQ : (q + 1) * HQ, :]
        ).reshape(T, C, P).astype(np.float16)
        in_maps.append({"x_seq": xs, "wmats": w6, "biases": b3})
    return in_maps


def kernel(video, Wz, bz, Wr, br, Wh, bh):
    _ensure_paths()
    from concourse.bass_utils import run_bass_kernel_spmd

    video = np.asarray(video, dtype=np.float32)
    nc = _get_prog()
    in_maps = _make_in_maps(video, Wz, bz, Wr, br, Wh, bh)
    res = run_bass_kernel_spmd(nc, in_maps, list(range(NCORES)))

    out = np.empty((B, T, C, H, W), np.float32)
    for core in range(NCORES):
        b_, q = divmod(core, 4)
        out[b_, :, :, q * HQ : (q + 1) * HQ, :] = np.asarray(
            res.results[core]["out_seq"]
        ).astype(np.float32).reshape(T, C, HQ, W)
    return out
